# revision 7
# baseline (speedup 1.0000x reference)
"""AttentionEdgeReadout kernel for 8 TRN2 NeuronCores.

Data-parallel over batch: B=8 batches -> one batch element per core.
Per core (all matmuls in fp32r = fp32 rounded to 11-bit mantissa, full PE rate):

  phase A: q1T = (Wq1*scale)^T X^T, k1T = Wk1^T X^T  (layout [D, N]),
           v1 = X Wv1                                (layout [N, D])
  phase B: per query-group g (4 groups of 512 queries):
           s1T[key, query] tiles -> exp -> eT (no transposes needed),
           att0T[d, q] = sum_k v1[k, d] eT[k, q]  + row-sums via ones-matmul,
           normalize columns by 1/rowsum (K=1 broadcast matmul + DVE mul),
           attT = Wo1^T att0T_norm,
           hT = leaky(W1^T attT + b1), threshneg_row = W2n^T hT + b2n (M=1 mm)
  phase C: q2T = (Wq2*scale)^T attT, k2T = Wk2^T attT
  phase D: per query tile (16 of 128): s2 stripe [128, 2048] in PSUM,
           surv = relu(s2 + threshneg) with free-axis accumulate -> denom,
           A = surv * 1/(denom + 1e-9), DMA out.

Self-contained: hardcodes B=8, N=2048, D=512, CF=128.
"""

import sys
import types

sys.path.insert(0, "/opt/trn_rl_repo")

import numpy as np

import concourse.bass as bass
import concourse.mybir as mybir
import concourse.tile as tile
from concourse import bacc
from concourse.bass_utils import run_bass_kernel_spmd

B, N, D, CF = 8, 2048, 512, 128
DC = D // 128     # 4 d-chunks
NT = N // 128     # 16 tiles
NG = N // 512     # 4 groups/blocks of 512
F32 = mybir.dt.float32
# Matmul operand dtype: native fp32 (4 cycles/row on PE, full precision).
# fp32r (1 cycle/row) was measured too imprecise for this problem: the
# thresholded readout chaotically amplifies the 2^-12 input rounding to
# ~4e-2 output rel error.
F32R = mybir.dt.float32
AF = mybir.ActivationFunctionType
ALU = mybir.AluOpType


def round_fp32r(x: np.ndarray) -> np.ndarray:
    """No-op in fp32 mode (kept for the fp32r experiment toggle)."""
    return np.ascontiguousarray(x, dtype=np.float32)


def build():
    nc = bacc.Bacc()

    XT_d = nc.declare_dram_parameter("XT", [D, N], F32R, isOutput=False)
    Wq1_d = nc.declare_dram_parameter("Wq1", [D, D], F32R, isOutput=False)
    Wk1_d = nc.declare_dram_parameter("Wk1", [D, D], F32R, isOutput=False)
    Wv1_d = nc.declare_dram_parameter("Wv1", [D, D], F32R, isOutput=False)
    Wo1_d = nc.declare_dram_parameter("Wo1", [D, D], F32R, isOutput=False)
    Wq2_d = nc.declare_dram_parameter("Wq2", [D, D], F32R, isOutput=False)
    Wk2_d = nc.declare_dram_parameter("Wk2", [D, D], F32R, isOutput=False)
    W1_d = nc.declare_dram_parameter("W1", [D, CF], F32R, isOutput=False)
    W2n_d = nc.declare_dram_parameter("W2n", [CF, 1], F32R, isOutput=False)
    b1r_d = nc.declare_dram_parameter("b1r", [1, CF], F32R, isOutput=False)
    b2n_d = nc.declare_dram_parameter("b2n", [1, 1], F32R, isOutput=False)
    ones_d = nc.declare_dram_parameter("ones", [128, 512], F32R, isOutput=False)
    out_d = nc.declare_dram_parameter("out", [N, N], F32, isOutput=True)

    def chunked(dram, free):
        # [D, free] dram -> [128, DC, free] sbuf view
        return dram.rearrange("(c p) n -> p c n", p=128)

    with tile.TileContext(nc) as tc:
        with (
            nc.allow_low_precision(
                reason="fp32r (11-bit mantissa) intermediates are intentional"
            ),
            tc.tile_pool(name="const", bufs=1) as const,
            tc.tile_pool(name="attTp", bufs=1) as attTp,
        ):
            ones = const.tile([128, 512], F32R)
            nc.sync.dma_start(ones[:], ones_d[:])
            W2n_s = const.tile([CF, 1], F32R)
            nc.sync.dma_start(W2n_s[:], W2n_d[:])
            b1r_s = const.tile([1, CF], F32R)
            nc.sync.dma_start(b1r_s[:], b1r_d[:])
            b2n_s = const.tile([1, 1], F32R)
            nc.sync.dma_start(b2n_s[:], b2n_d[:])
            Wo1_s = const.tile([128, DC, D], F32R)
            nc.sync.dma_start(Wo1_s[:], chunked(Wo1_d, D))
            W1_s = const.tile([128, DC, CF], F32R)
            nc.sync.dma_start(W1_s[:], chunked(W1_d, CF))
            threshneg = const.tile([128, NT], F32)

            attT = attTp.tile([128, DC, N], F32R)

            with tc.tile_pool(name="acts", bufs=1) as acts:
                q1T = acts.tile([128, DC, N], F32R)
                k1T = acts.tile([128, DC, N], F32R)
                v1 = acts.tile([128, NT, D], F32R)

                # ---------------- phase A ----------------
                with (
                    tc.tile_pool(name="wA", bufs=1) as wA,
                    tc.tile_pool(name="xt", bufs=2) as xtp,
                    tc.tile_pool(name="psA", bufs=4, space="PSUM") as psA,
                ):
                    Wq1_s = wA.tile([128, DC, D], F32R)
                    nc.sync.dma_start(Wq1_s[:], chunked(Wq1_d, D))
                    Wk1_s = wA.tile([128, DC, D], F32R)
                    nc.sync.dma_start(Wk1_s[:], chunked(Wk1_d, D))
                    Wv1_s = wA.tile([128, DC, D], F32R)
                    nc.sync.dma_start(Wv1_s[:], chunked(Wv1_d, D))

                    for b in range(NG):
                        xt = xtp.tile([128, DC, 512], F32R)
                        nc.sync.dma_start(
                            xt[:],
                            XT_d[:, b * 512 : (b + 1) * 512].rearrange(
                                "(c p) n -> p c n", p=128
                            ),
                        )
                        # q1T / k1T chunks for this block
                        for W_s, dst in ((Wq1_s, q1T), (Wk1_s, k1T)):
                            for c in range(DC):
                                ps = psA.tile([128, 512], F32, tag="psA")
                                for dc in range(DC):
                                    nc.tensor.matmul(
                                        ps[:],
                                        W_s[:, dc, c * 128 : (c + 1) * 128],
                                        xt[:, dc, :],
                                        start=(dc == 0),
                                        stop=(dc == DC - 1),
                                    )
                                nc.vector.tensor_copy(
                                    dst[:, c, b * 512 : (b + 1) * 512], ps[:]
                                )
                        # v1 tiles for this block
                        for lt in range(4):
                            t = b * 4 + lt
                            ps = psA.tile([128, 512], F32, tag="psA")
                            for dc in range(DC):
                                nc.tensor.matmul(
                                    ps[:],
                                    xt[:, dc, lt * 128 : (lt + 1) * 128],
                                    Wv1_s[:, dc, :],
                                    start=(dc == 0),
                                    stop=(dc == DC - 1),
                                )
                            nc.vector.tensor_copy(v1[:, t, :], ps[:])

                # ---------------- phase B ----------------
                with (
                    tc.tile_pool(name="phB", bufs=1) as phB,
                    tc.tile_pool(name="etp", bufs=3) as etp,
                    tc.tile_pool(name="psS", bufs=4, space="PSUM") as psS,
                    tc.tile_pool(name="psAcc", bufs=1, space="PSUM") as psAcc,
                ):
                    for g in range(NG):
                        gs = slice(g * 512, (g + 1) * 512)
                        acc = psAcc.tile([128, DC, 512], F32, tag="acc")
                        rs = psS.tile([128, 512], F32, tag="psS")
                        for t in range(NT):
                            ps = psS.tile([128, 512], F32, tag="psS")
                            for dc in range(DC):
                                nc.tensor.matmul(
                                    ps[:],
                                    k1T[:, dc, t * 128 : (t + 1) * 128],
                                    q1T[:, dc, gs],
                                    start=(dc == 0),
                                    stop=(dc == DC - 1),
                                )
                            et = etp.tile([128, 512], F32R, tag="et")
                            nc.scalar.activation(et[:], ps[:], AF.Exp)
                            # accumulate att0T (4 chunks) + row-sums
                            for c in range(DC):
                                nc.tensor.matmul(
                                    acc[:, c, :],
                                    v1[:, t, c * 128 : (c + 1) * 128],
                                    et[:],
                                    start=(t == 0),
                                    stop=(t == NT - 1),
                                )
                            nc.tensor.matmul(
                                rs[0:1, :],
                                ones[:, 0:1],
                                et[:],
                                start=(t == 0),
                                stop=(t == NT - 1),
                            )
                        # 1/rowsum -> broadcast to 128 partitions via K=1 matmul
                        recip = phB.tile([1, 512], F32R, tag="recip")
                        nc.vector.reciprocal(recip[:], rs[0:1, :])
                        rbc_ps = psS.tile([128, 512], F32, tag="psS")
                        nc.tensor.matmul(
                            rbc_ps[:], ones[0:1, 0:128], recip[:], start=True, stop=True
                        )
                        rbc = phB.tile([128, 512], F32, tag="rbc")
                        nc.vector.tensor_copy(rbc[:], rbc_ps[:])
                        # normalized att0T
                        att0T = phB.tile([128, DC, 512], F32R, tag="att0T")
                        for c in range(DC):
                            nc.vector.tensor_tensor(
                                att0T[:, c, :], acc[:, c, :], rbc[:], ALU.mult
                            )
                        # attT[:, c, gs] = Wo1^T @ att0T
                        for c in range(DC):
                            ps = psS.tile([128, 512], F32, tag="psS")
                            for dc in range(DC):
                                nc.tensor.matmul(
                                    ps[:],
                                    Wo1_s[:, dc, c * 128 : (c + 1) * 128],
                                    att0T[:, dc, :],
                                    start=(dc == 0),
                                    stop=(dc == DC - 1),
                                )
                            nc.vector.tensor_copy(attT[:, c, gs], ps[:])
                        # hT = leaky(W1^T attT + b1)
                        ps_h = psS.tile([128, 512], F32, tag="psS")
                        for dc in range(DC):
                            nc.tensor.matmul(
                                ps_h[:],
                                W1_s[:, dc, :],
                                attT[:, dc, gs],
                                start=(dc == 0),
                                stop=False,
                            )
                        nc.tensor.matmul(
                            ps_h[:], b1r_s[:], ones[0:1, :], start=False, stop=True
                        )
                        htmp = phB.tile([128, 512], F32, tag="htmp")
                        nc.vector.tensor_scalar_mul(htmp[:], ps_h[:], 0.1)
                        hT = phB.tile([128, 512], F32R, tag="hT")
                        nc.vector.tensor_tensor(hT[:], ps_h[:], htmp[:], ALU.max)
                        # threshneg row = W2n^T hT + b2n   [1, 512]
                        ps_t = psS.tile([128, 512], F32, tag="psS")
                        nc.tensor.matmul(
                            ps_t[0:1, :], W2n_s[:], hT[:], start=True, stop=False
                        )
                        nc.tensor.matmul(
                            ps_t[0:1, :], b2n_s[:], ones[0:1, :], start=False, stop=True
                        )
                        trow = phB.tile([1, 512], F32, tag="trow")
                        nc.vector.tensor_copy(trow[:], ps_t[0:1, :])
                        # scatter [1, 512] -> threshneg[:, 4 tiles] columns
                        for lt in range(4):
                            nc.sync.dma_start(
                                threshneg[:, g * 4 + lt : g * 4 + lt + 1],
                                trow[0:1, lt * 128 : (lt + 1) * 128],
                            )

            # ---------------- phase C ----------------
            with tc.tile_pool(name="qk2", bufs=1) as qk2:
                Wq2_s = qk2.tile([128, DC, D], F32R)
                nc.sync.dma_start(Wq2_s[:], chunked(Wq2_d, D))
                Wk2_s = qk2.tile([128, DC, D], F32R)
                nc.sync.dma_start(Wk2_s[:], chunked(Wk2_d, D))
                q2T = qk2.tile([128, DC, N], F32R)
                k2T = qk2.tile([128, DC, N], F32R)
                with tc.tile_pool(name="psC", bufs=4, space="PSUM") as psC:
                    for b in range(NG):
                        bs = slice(b * 512, (b + 1) * 512)
                        for W_s, dst in ((Wq2_s, q2T), (Wk2_s, k2T)):
                            for c in range(DC):
                                ps = psC.tile([128, 512], F32, tag="psC")
                                for dc in range(DC):
                                    nc.tensor.matmul(
                                        ps[:],
                                        W_s[:, dc, c * 128 : (c + 1) * 128],
                                        attT[:, dc, bs],
                                        start=(dc == 0),
                                        stop=(dc == DC - 1),
                                    )
                                nc.vector.tensor_copy(dst[:, c, bs], ps[:])

                # ---------------- phase D ----------------
                with (
                    tc.tile_pool(name="phD", bufs=2) as phD,
                    tc.tile_pool(name="psD", bufs=2, space="PSUM") as psD,
                ):
                    for qt in range(NT):
                        s2 = psD.tile([128, NG, 512], F32, tag="s2")
                        for b in range(NG):
                            for dc in range(DC):
                                nc.tensor.matmul(
                                    s2[:, b, :],
                                    q2T[:, dc, qt * 128 : (qt + 1) * 128],
                                    k2T[:, dc, b * 512 : (b + 1) * 512],
                                    start=(dc == 0),
                                    stop=(dc == DC - 1),
                                )
                        surv = phD.tile([128, NG, 512], F32, tag="surv")
                        denom = phD.tile([128, 1], F32, tag="denom")
                        nc.scalar.activation(
                            surv[:],
                            s2[:],
                            AF.Relu,
                            bias=threshneg[:, qt : qt + 1],
                            accum_out=denom[:],
                        )
                        recip = phD.tile([128, 1], F32, tag="recipD")
                        nc.vector.tensor_scalar_add(denom[:], denom[:], 1e-9)
                        nc.vector.reciprocal(recip[:], denom[:])
                        ot = phD.tile([128, NG, 512], F32, tag="ot")
                        nc.vector.tensor_scalar_mul(ot[:], surv[:], recip[:])
                        nc.sync.dma_start(
                            out_d[qt * 128 : (qt + 1) * 128, :],
                            ot.rearrange("p b n -> p (b n)"),
                        )

    nc.finalize()
    return nc


_NC = None


def _get_nc():
    global _NC
    if _NC is None:
        _NC = build()
    return _NC


def make_in_maps(X, Wq1, Wk1, Wv1, Wo1, Wq2, Wk2, W1, b1, W2, b2):
    scale = np.float32(1.0 / np.sqrt(np.float32(D)))
    shared = {
        "Wq1": round_fp32r(np.asarray(Wq1) * scale),
        "Wk1": round_fp32r(Wk1),
        "Wv1": round_fp32r(Wv1),
        "Wo1": round_fp32r(Wo1),
        "Wq2": round_fp32r(np.asarray(Wq2) * scale),
        "Wk2": round_fp32r(Wk2),
        "W1": round_fp32r(W1),
        "W2n": round_fp32r(-np.asarray(W2).reshape(CF, 1)),
        "b1r": round_fp32r(np.asarray(b1).reshape(1, CF)),
        "b2n": round_fp32r(-np.asarray(b2).reshape(1, 1)),
        "ones": np.ones((128, 512), dtype=np.float32),
    }
    X = np.asarray(X, dtype=np.float32)
    return [
        {"XT": round_fp32r(np.ascontiguousarray(X[i].T)), **shared} for i in range(B)
    ]


def run(in_maps, trace=False, **kwargs):
    return run_bass_kernel_spmd(
        _get_nc(), in_maps, core_ids=list(range(B)), trace=trace, **kwargs
    )


def kernel(X, Wq1, Wk1, Wv1, Wo1, Wq2, Wk2, W1, b1, W2, b2):
    in_maps = make_in_maps(X, Wq1, Wk1, Wv1, Wo1, Wq2, Wk2, W1, b1, W2, b2)
    res = run(in_maps, trace=False)
    return np.stack([res.results[i]["out"] for i in range(B)], axis=0)


# revision 11
# speedup vs baseline: 1.0825x; 1.0825x over previous
"""AttentionEdgeReadout kernel for 8 TRN2 NeuronCores.

Data-parallel over batch: B=8 batches -> one batch element per core.

Precision scheme: every matmul operand X is carried as a bf16 pair
(Xh = bf16(X), Xl = bf16(X - Xh), ~16-bit effective mantissa) and each
logical matmul runs as 3 full-rate bf16 passes accumulated in fp32 PSUM:
  X @ Y ~= Xh@Yh + Xh@Yl + Xl@Yh          (Xl@Yl ~ 2^-18 rel, dropped)
This is ~3 PE cycles/row vs native fp32's effective ~5.7 cycles/row, and
gives ~4e-4 output rel error (the thresholded readout chaotically
amplifies operand rounding; plain bf16 or fp32r operands fail the 2e-2
accuracy gate, measured 4e-2 for fp32r).

Per core:
  phase A: q1T = (Wq1*scale)^T X^T, k1T = Wk1^T X^T  (layout [D, N]),
           v1 = X Wv1                                (layout [N, D])
  phase B: per query-group g (4 groups of 512 queries):
           s1T[key, query] tiles -> exp -> eT (no PE transposes needed),
           att0T[d, q] = sum_k v1[k, d] eT[k, q], row-sums via ones-matmul,
           normalize columns by 1/rowsum (K=1 broadcast matmul + DVE mul),
           attT = Wo1^T att0T_norm,
           hT = leaky(W1^T attT + b1), threshneg_row = W2n^T hT + b2n (M=1)
  phase C: q2T = (Wq2*scale)^T attT, k2T = Wk2^T attT
  phase D: per query tile (16 of 128): s2 stripe [128, 2048] in PSUM,
           surv = relu(s2 + threshneg) with free-axis accumulate -> denom,
           A = surv * 1/(denom + 1e-9), DMA out.

Self-contained: hardcodes B=8, N=2048, D=512, CF=128.
"""

import sys

sys.path.insert(0, "/opt/trn_rl_repo")

import ml_dtypes
import numpy as np

import concourse.bass as bass
import concourse.mybir as mybir
import concourse.tile as tile
from concourse import bacc
from concourse.bass_utils import run_bass_kernel_spmd

B, N, D, CF = 8, 2048, 512, 128
DC = D // 128     # 4 d-chunks
NT = N // 128     # 16 tiles
NG = N // 512     # 4 groups/blocks of 512
F32 = mybir.dt.float32
BF16 = mybir.dt.bfloat16
AF = mybir.ActivationFunctionType
ALU = mybir.AluOpType
BFNP = ml_dtypes.bfloat16


def split_pair(x):
    """Host-side bf16 hi/lo split of an fp32 array."""
    x = np.ascontiguousarray(x, dtype=np.float32)
    xh = x.astype(BFNP)
    xl = (x - xh.astype(np.float32)).astype(BFNP)
    return xh, xl


def build():
    nc = bacc.Bacc()

    def pair_param(name, shape):
        return (
            nc.declare_dram_parameter(name + "h", shape, BF16, isOutput=False),
            nc.declare_dram_parameter(name + "l", shape, BF16, isOutput=False),
        )

    XT_d = pair_param("XT", [D, N])
    Wq1_d = pair_param("Wq1", [D, D])
    Wk1_d = pair_param("Wk1", [D, D])
    Wv1_d = pair_param("Wv1", [D, D])
    Wo1_d = pair_param("Wo1", [D, D])
    Wq2_d = pair_param("Wq2", [D, D])
    Wk2_d = pair_param("Wk2", [D, D])
    W1_d = pair_param("W1", [D, CF])
    W2n_d = pair_param("W2n", [CF, 1])
    b1r_d = pair_param("b1r", [1, CF])
    b2n_d = pair_param("b2n", [1, 1])
    ones_d = nc.declare_dram_parameter("ones", [128, 512], BF16, isOutput=False)
    out_d = nc.declare_dram_parameter("out", [N, N], F32, isOutput=True)

    def chunked(dram):
        return dram.rearrange("(c p) n -> p c n", p=128)

    def mm3(ps, ah, al, bh, bl, start, stop):
        """ps += a @ b with a, b bf16 hi/lo split (3 passes)."""
        nc.tensor.matmul(ps, ah, bh, start=start, stop=False)
        nc.tensor.matmul(ps, ah, bl, start=False, stop=False)
        nc.tensor.matmul(ps, al, bh, start=False, stop=stop)

    def split_evac(hi, lo, src):
        """hi = bf16(src); lo = bf16(src - hi) on DVE."""
        nc.vector.tensor_copy(hi, src)
        nc.vector.tensor_sub(lo, src, hi)

    with tile.TileContext(nc) as tc:
        with (
            nc.allow_low_precision(reason="bf16 hi/lo split pairs are intentional"),
            tc.tile_pool(name="const", bufs=1) as const,
            tc.tile_pool(name="attTp", bufs=1) as attTp,
        ):
            def pair_tile(pool, name, shape):
                return (
                    pool.tile(shape, BF16, tag=name + "h", name=name + "h"),
                    pool.tile(shape, BF16, tag=name + "l", name=name + "l"),
                )

            def pair_load(pool, name, dram_pair, shape, view=None):
                ts_ = []
                for d_, sfx in zip(dram_pair, "hl"):
                    t = pool.tile(shape, BF16, tag=name + sfx, name=name + sfx)
                    nc.sync.dma_start(t[:], view(d_) if view else d_[:])
                    ts_.append(t)
                return ts_

            ones = const.tile([128, 512], BF16)
            nc.sync.dma_start(ones[:], ones_d[:])
            W2n_s = pair_load(const, "W2n", W2n_d, [CF, 1])
            b1r_s = pair_load(const, "b1r", b1r_d, [1, CF])
            b2n_s = pair_load(const, "b2n", b2n_d, [1, 1])
            Wo1_s = pair_load(const, "Wo1", Wo1_d, [128, DC, D], chunked)
            W1_s = pair_load(const, "W1", W1_d, [128, DC, CF], chunked)
            threshneg = const.tile([128, NT], F32)

            attTh, attTl = pair_tile(attTp, "attT", [128, DC, N])

            with tc.tile_pool(name="acts", bufs=1) as acts:
                q1Th, q1Tl = pair_tile(acts, "q1T", [128, DC, N])
                k1Th, k1Tl = pair_tile(acts, "k1T", [128, DC, N])
                v1h, v1l = pair_tile(acts, "v1", [128, NT, D])

                # ---------------- phase A ----------------
                with (
                    tc.tile_pool(name="wA", bufs=1) as wA,
                    tc.tile_pool(name="xt", bufs=2) as xtp,
                    tc.tile_pool(name="psA", bufs=4, space="PSUM") as psA,
                ):
                    Wq1_s = pair_load(wA, "Wq1", Wq1_d, [128, DC, D], chunked)
                    Wk1_s = pair_load(wA, "Wk1", Wk1_d, [128, DC, D], chunked)
                    Wv1_s = pair_load(wA, "Wv1", Wv1_d, [128, DC, D], chunked)

                    for b in range(NG):
                        xth = xtp.tile([128, DC, 512], BF16, tag="xth")
                        xtl = xtp.tile([128, DC, 512], BF16, tag="xtl")
                        for t_, d_ in ((xth, XT_d[0]), (xtl, XT_d[1])):
                            nc.sync.dma_start(
                                t_[:],
                                d_[:, b * 512 : (b + 1) * 512].rearrange(
                                    "(c p) n -> p c n", p=128
                                ),
                            )
                        for W_s, (dsth, dstl) in (
                            (Wq1_s, (q1Th, q1Tl)),
                            (Wk1_s, (k1Th, k1Tl)),
                        ):
                            for c in range(DC):
                                ps = psA.tile([128, 512], F32, tag="psA")
                                for dc in range(DC):
                                    cs = slice(c * 128, (c + 1) * 128)
                                    mm3(
                                        ps[:],
                                        W_s[0][:, dc, cs],
                                        W_s[1][:, dc, cs],
                                        xth[:, dc, :],
                                        xtl[:, dc, :],
                                        start=(dc == 0),
                                        stop=(dc == DC - 1),
                                    )
                                bs = slice(b * 512, (b + 1) * 512)
                                split_evac(dsth[:, c, bs], dstl[:, c, bs], ps[:])
                        for lt in range(4):
                            t = b * 4 + lt
                            ps = psA.tile([128, 512], F32, tag="psA")
                            for dc in range(DC):
                                ls = slice(lt * 128, (lt + 1) * 128)
                                mm3(
                                    ps[:],
                                    xth[:, dc, ls],
                                    xtl[:, dc, ls],
                                    Wv1_s[0][:, dc, :],
                                    Wv1_s[1][:, dc, :],
                                    start=(dc == 0),
                                    stop=(dc == DC - 1),
                                )
                            split_evac(v1h[:, t, :], v1l[:, t, :], ps[:])

                # ---------------- phase B ----------------
                with (
                    tc.tile_pool(name="phB", bufs=1) as phB,
                    tc.tile_pool(name="etp", bufs=3) as etp,
                    tc.tile_pool(name="psS", bufs=4, space="PSUM") as psS,
                    tc.tile_pool(name="psAcc", bufs=1, space="PSUM") as psAcc,
                ):
                    for g in range(NG):
                        gs = slice(g * 512, (g + 1) * 512)
                        acc = psAcc.tile([128, DC, 512], F32, tag="acc")
                        rs = psS.tile([128, 512], F32, tag="psS")
                        for t in range(NT):
                            ps = psS.tile([128, 512], F32, tag="psS")
                            for dc in range(DC):
                                ks = slice(t * 128, (t + 1) * 128)
                                mm3(
                                    ps[:],
                                    k1Th[:, dc, ks],
                                    k1Tl[:, dc, ks],
                                    q1Th[:, dc, gs],
                                    q1Tl[:, dc, gs],
                                    start=(dc == 0),
                                    stop=(dc == DC - 1),
                                )
                            ef = etp.tile([128, 512], F32, tag="ef")
                            nc.scalar.activation(ef[:], ps[:], AF.Exp)
                            eh = etp.tile([128, 512], BF16, tag="eh")
                            el = etp.tile([128, 512], BF16, tag="el")
                            split_evac(eh[:], el[:], ef[:])
                            for c in range(DC):
                                cs = slice(c * 128, (c + 1) * 128)
                                mm3(
                                    acc[:, c, :],
                                    v1h[:, t, cs],
                                    v1l[:, t, cs],
                                    eh[:],
                                    el[:],
                                    start=(t == 0),
                                    stop=(t == NT - 1),
                                )
                            # row-sums: ones is exact in bf16, 2 passes
                            nc.tensor.matmul(
                                rs[0:1, :], ones[:, 0:1], eh[:],
                                start=(t == 0), stop=False,
                            )
                            nc.tensor.matmul(
                                rs[0:1, :], ones[:, 0:1], el[:],
                                start=False, stop=(t == NT - 1),
                            )
                        # 1/rowsum, split, broadcast to 128 partitions via K=1 mm
                        recf = phB.tile([1, 512], F32, tag="recf")
                        nc.vector.reciprocal(recf[:], rs[0:1, :])
                        rech = phB.tile([1, 512], BF16, tag="rech")
                        recl = phB.tile([1, 512], BF16, tag="recl")
                        split_evac(rech[:], recl[:], recf[:])
                        rbc_ps = psS.tile([128, 512], F32, tag="psS")
                        nc.tensor.matmul(
                            rbc_ps[:], ones[0:1, 0:128], rech[:], start=True, stop=False
                        )
                        nc.tensor.matmul(
                            rbc_ps[:], ones[0:1, 0:128], recl[:], start=False, stop=True
                        )
                        rbc = phB.tile([128, 512], F32, tag="rbc")
                        nc.vector.tensor_copy(rbc[:], rbc_ps[:])
                        # normalized att0T (hi/lo)
                        a0f = phB.tile([128, DC, 512], F32, tag="a0f")
                        a0h = phB.tile([128, DC, 512], BF16, tag="a0h")
                        a0l = phB.tile([128, DC, 512], BF16, tag="a0l")
                        for c in range(DC):
                            nc.vector.tensor_mul(a0f[:, c, :], acc[:, c, :], rbc[:])
                            split_evac(a0h[:, c, :], a0l[:, c, :], a0f[:, c, :])
                        # attT[:, c, gs] = Wo1^T @ att0T
                        for c in range(DC):
                            ps = psS.tile([128, 512], F32, tag="psS")
                            for dc in range(DC):
                                cs = slice(c * 128, (c + 1) * 128)
                                mm3(
                                    ps[:],
                                    Wo1_s[0][:, dc, cs],
                                    Wo1_s[1][:, dc, cs],
                                    a0h[:, dc, :],
                                    a0l[:, dc, :],
                                    start=(dc == 0),
                                    stop=(dc == DC - 1),
                                )
                            split_evac(attTh[:, c, gs], attTl[:, c, gs], ps[:])
                        # hT = leaky(W1^T attT + b1)
                        ps_h = psS.tile([128, 512], F32, tag="psS")
                        for dc in range(DC):
                            mm3(
                                ps_h[:],
                                W1_s[0][:, dc, :],
                                W1_s[1][:, dc, :],
                                attTh[:, dc, gs],
                                attTl[:, dc, gs],
                                start=(dc == 0),
                                stop=False,
                            )
                        nc.tensor.matmul(
                            ps_h[:], b1r_s[0][:], ones[0:1, :], start=False, stop=False
                        )
                        nc.tensor.matmul(
                            ps_h[:], b1r_s[1][:], ones[0:1, :], start=False, stop=True
                        )
                        htmp = phB.tile([128, 512], F32, tag="htmp")
                        nc.vector.tensor_scalar_mul(htmp[:], ps_h[:], 0.1)
                        hf = phB.tile([128, 512], F32, tag="hf")
                        nc.vector.tensor_tensor(hf[:], ps_h[:], htmp[:], ALU.max)
                        hTh = phB.tile([128, 512], BF16, tag="hTh")
                        hTl = phB.tile([128, 512], BF16, tag="hTl")
                        split_evac(hTh[:], hTl[:], hf[:])
                        # threshneg row = W2n^T hT + b2n   [1, 512]
                        ps_t = psS.tile([128, 512], F32, tag="psS")
                        mm3(
                            ps_t[0:1, :], W2n_s[0][:], W2n_s[1][:], hTh[:], hTl[:],
                            start=True, stop=False,
                        )
                        nc.tensor.matmul(
                            ps_t[0:1, :], b2n_s[0][:], ones[0:1, :],
                            start=False, stop=False,
                        )
                        nc.tensor.matmul(
                            ps_t[0:1, :], b2n_s[1][:], ones[0:1, :],
                            start=False, stop=True,
                        )
                        trow = phB.tile([1, 512], F32, tag="trow")
                        nc.vector.tensor_copy(trow[:], ps_t[0:1, :])
                        for lt in range(4):
                            nc.sync.dma_start(
                                threshneg[:, g * 4 + lt : g * 4 + lt + 1],
                                trow[0:1, lt * 128 : (lt + 1) * 128],
                            )

            # ---------------- phase C ----------------
            with tc.tile_pool(name="qk2", bufs=1) as qk2:
                Wq2_s = pair_load(qk2, "Wq2", Wq2_d, [128, DC, D], chunked)
                Wk2_s = pair_load(qk2, "Wk2", Wk2_d, [128, DC, D], chunked)
                q2Th, q2Tl = pair_tile(qk2, "q2T", [128, DC, N])
                k2Th, k2Tl = pair_tile(qk2, "k2T", [128, DC, N])
                with tc.tile_pool(name="psC", bufs=4, space="PSUM") as psC:
                    for b in range(NG):
                        bs = slice(b * 512, (b + 1) * 512)
                        for W_s, (dsth, dstl) in (
                            (Wq2_s, (q2Th, q2Tl)),
                            (Wk2_s, (k2Th, k2Tl)),
                        ):
                            for c in range(DC):
                                ps = psC.tile([128, 512], F32, tag="psC")
                                for dc in range(DC):
                                    cs = slice(c * 128, (c + 1) * 128)
                                    mm3(
                                        ps[:],
                                        W_s[0][:, dc, cs],
                                        W_s[1][:, dc, cs],
                                        attTh[:, dc, bs],
                                        attTl[:, dc, bs],
                                        start=(dc == 0),
                                        stop=(dc == DC - 1),
                                    )
                                split_evac(dsth[:, c, bs], dstl[:, c, bs], ps[:])

                # ---------------- phase D ----------------
                with (
                    tc.tile_pool(name="phD", bufs=2) as phD,
                    tc.tile_pool(name="psD", bufs=2, space="PSUM") as psD,
                ):
                    for qt in range(NT):
                        qs = slice(qt * 128, (qt + 1) * 128)
                        s2 = psD.tile([128, NG, 512], F32, tag="s2")
                        for b in range(NG):
                            for dc in range(DC):
                                mm3(
                                    s2[:, b, :],
                                    q2Th[:, dc, qs],
                                    q2Tl[:, dc, qs],
                                    k2Th[:, dc, b * 512 : (b + 1) * 512],
                                    k2Tl[:, dc, b * 512 : (b + 1) * 512],
                                    start=(dc == 0),
                                    stop=(dc == DC - 1),
                                )
                        surv = phD.tile([128, NG, 512], F32, tag="surv")
                        denom = phD.tile([128, 1], F32, tag="denom")
                        nc.scalar.activation(
                            surv[:],
                            s2[:],
                            AF.Relu,
                            bias=threshneg[:, qt : qt + 1],
                            accum_out=denom[:],
                        )
                        recip = phD.tile([128, 1], F32, tag="recipD")
                        nc.vector.tensor_scalar_add(denom[:], denom[:], 1e-9)
                        nc.vector.reciprocal(recip[:], denom[:])
                        ot = phD.tile([128, NG, 512], F32, tag="ot")
                        nc.vector.tensor_scalar_mul(ot[:], surv[:], recip[:])
                        nc.sync.dma_start(
                            out_d[qs, :], ot.rearrange("p b n -> p (b n)")
                        )

    nc.finalize()
    return nc


_NC = None


def _get_nc():
    global _NC
    if _NC is None:
        _NC = build()
    return _NC


def make_in_maps(X, Wq1, Wk1, Wv1, Wo1, Wq2, Wk2, W1, b1, W2, b2):
    scale = np.float32(1.0 / np.sqrt(np.float32(D)))
    shared = {}

    def add_pair(name, x):
        h, l = split_pair(x)
        shared[name + "h"] = h
        shared[name + "l"] = l

    add_pair("Wq1", np.asarray(Wq1, np.float32) * scale)
    add_pair("Wk1", Wk1)
    add_pair("Wv1", Wv1)
    add_pair("Wo1", Wo1)
    add_pair("Wq2", np.asarray(Wq2, np.float32) * scale)
    add_pair("Wk2", Wk2)
    add_pair("W1", W1)
    add_pair("W2n", -np.asarray(W2, np.float32).reshape(CF, 1))
    add_pair("b1r", np.asarray(b1, np.float32).reshape(1, CF))
    add_pair("b2n", -np.asarray(b2, np.float32).reshape(1, 1))
    shared["ones"] = np.ones((128, 512), dtype=BFNP)

    X = np.asarray(X, dtype=np.float32)
    in_maps = []
    for i in range(B):
        xh, xl = split_pair(X[i].T)
        in_maps.append({"XTh": xh, "XTl": xl, **shared})
    return in_maps


def run(in_maps, trace=False, **kwargs):
    return run_bass_kernel_spmd(
        _get_nc(), in_maps, core_ids=list(range(B)), trace=trace, **kwargs
    )


def kernel(X, Wq1, Wk1, Wv1, Wo1, Wq2, Wk2, W1, b1, W2, b2):
    in_maps = make_in_maps(X, Wq1, Wk1, Wv1, Wo1, Wq2, Wk2, W1, b1, W2, b2)
    res = run(in_maps, trace=False)
    return np.stack([res.results[i]["out"] for i in range(B)], axis=0)


# revision 12
# speedup vs baseline: 1.1365x; 1.0499x over previous
"""AttentionEdgeReadout kernel for 8 TRN2 NeuronCores.

Data-parallel over batch: B=8 batches -> one batch element per core.

Precision scheme: every matmul operand X is carried as a bf16 pair
(Xh = bf16(X), Xl = bf16(X - Xh), ~16-bit effective mantissa) and each
logical matmul runs as 3 full-rate bf16 passes accumulated in fp32 PSUM:
  X @ Y ~= Xh@Yh + Xh@Yl + Xl@Yh          (Xl@Yl ~ 2^-18 rel, dropped)
This is ~3 PE cycles/row vs native fp32's effective ~5.7 cycles/row, and
gives ~4e-4 output rel error (the thresholded readout chaotically
amplifies operand rounding; plain bf16 or fp32r operands fail the 2e-2
accuracy gate, measured 4e-2 for fp32r).

Per core:
  phase A: q1T = (Wq1*scale)^T X^T, k1T = Wk1^T X^T  (layout [D, N]),
           v1 = X Wv1                                (layout [N, D])
  phase B: per query-group g (4 groups of 512 queries):
           s1T[key, query] tiles -> exp -> eT (no PE transposes needed),
           att0T[d, q] = sum_k v1[k, d] eT[k, q], row-sums via ones-matmul,
           normalize columns by 1/rowsum (K=1 broadcast matmul + DVE mul),
           attT = Wo1^T att0T_norm,
           hT = leaky(W1^T attT + b1), threshneg_row = W2n^T hT + b2n (M=1)
  phase C: q2T = (Wq2*scale)^T attT, k2T = Wk2^T attT
  phase D: per query tile (16 of 128): s2 stripe [128, 2048] in PSUM,
           surv = relu(s2 + threshneg) with free-axis accumulate -> denom,
           A = surv * 1/(denom + 1e-9), DMA out.

Self-contained: hardcodes B=8, N=2048, D=512, CF=128.
"""

import sys

sys.path.insert(0, "/opt/trn_rl_repo")

import ml_dtypes
import numpy as np

import concourse.bass as bass
import concourse.mybir as mybir
import concourse.tile as tile
from concourse import bacc
from concourse.bass_utils import run_bass_kernel_spmd

B, N, D, CF = 8, 2048, 512, 128
DC = D // 128     # 4 d-chunks
NT = N // 128     # 16 tiles
NG = N // 512     # 4 groups/blocks of 512
F32 = mybir.dt.float32
BF16 = mybir.dt.bfloat16
AF = mybir.ActivationFunctionType
ALU = mybir.AluOpType
BFNP = ml_dtypes.bfloat16


def split_pair(x):
    """Host-side bf16 hi/lo split of an fp32 array."""
    x = np.ascontiguousarray(x, dtype=np.float32)
    xh = x.astype(BFNP)
    xl = (x - xh.astype(np.float32)).astype(BFNP)
    return xh, xl


def build():
    nc = bacc.Bacc()

    def pair_param(name, shape):
        return (
            nc.declare_dram_parameter(name + "h", shape, BF16, isOutput=False),
            nc.declare_dram_parameter(name + "l", shape, BF16, isOutput=False),
        )

    XT_d = pair_param("XT", [D, N])
    Wq1_d = pair_param("Wq1", [D, D])
    Wk1_d = pair_param("Wk1", [D, D])
    Wv1_d = pair_param("Wv1", [D, D])
    Wo1_d = pair_param("Wo1", [D, D])
    Wq2_d = pair_param("Wq2", [D, D])
    Wk2_d = pair_param("Wk2", [D, D])
    W1_d = pair_param("W1", [D, CF])
    W2n_d = pair_param("W2n", [CF, 1])
    b1r_d = pair_param("b1r", [1, CF])
    b2n_d = pair_param("b2n", [1, 1])
    ones_d = nc.declare_dram_parameter("ones", [128, 512], BF16, isOutput=False)
    out_d = nc.declare_dram_parameter("out", [N, N], F32, isOutput=True)

    def chunked(dram):
        return dram.rearrange("(c p) n -> p c n", p=128)

    def mm3(ps, ah, al, bh, bl, start, stop):
        """ps += a @ b with a, b bf16 hi/lo split (3 passes)."""
        nc.tensor.matmul(ps, ah, bh, start=start, stop=False)
        nc.tensor.matmul(ps, ah, bl, start=False, stop=False)
        nc.tensor.matmul(ps, al, bh, start=False, stop=stop)

    def split_evac(hi, lo, src):
        """hi = bf16(src); lo = bf16(src - hi) on DVE."""
        nc.vector.tensor_copy(hi, src)
        nc.vector.tensor_sub(lo, src, hi)

    with tile.TileContext(nc) as tc:
        with (
            nc.allow_low_precision(reason="bf16 hi/lo split pairs are intentional"),
            tc.tile_pool(name="const", bufs=1) as const,
            tc.tile_pool(name="attTp", bufs=1) as attTp,
        ):
            def pair_tile(pool, name, shape):
                return (
                    pool.tile(shape, BF16, tag=name + "h", name=name + "h"),
                    pool.tile(shape, BF16, tag=name + "l", name=name + "l"),
                )

            def pair_load(pool, name, dram_pair, shape, view=None):
                ts_ = []
                for d_, sfx in zip(dram_pair, "hl"):
                    t = pool.tile(shape, BF16, tag=name + sfx, name=name + sfx)
                    nc.sync.dma_start(t[:], view(d_) if view else d_[:])
                    ts_.append(t)
                return ts_

            ones = const.tile([128, 512], BF16)
            nc.sync.dma_start(ones[:], ones_d[:])
            W2n_s = pair_load(const, "W2n", W2n_d, [CF, 1])
            b1r_s = pair_load(const, "b1r", b1r_d, [1, CF])
            b2n_s = pair_load(const, "b2n", b2n_d, [1, 1])
            Wo1_s = pair_load(const, "Wo1", Wo1_d, [128, DC, D], chunked)
            W1_s = pair_load(const, "W1", W1_d, [128, DC, CF], chunked)
            threshneg = const.tile([128, NT], F32)

            attTh, attTl = pair_tile(attTp, "attT", [128, DC, N])

            with tc.tile_pool(name="acts", bufs=1) as acts:
                q1Th, q1Tl = pair_tile(acts, "q1T", [128, DC, N])
                k1Th, k1Tl = pair_tile(acts, "k1T", [128, DC, N])
                v1h, v1l = pair_tile(acts, "v1", [128, NT, D])

                # ---------------- phase A ----------------
                with (
                    tc.tile_pool(name="wA", bufs=1) as wA,
                    tc.tile_pool(name="xt", bufs=2) as xtp,
                    tc.tile_pool(name="psA", bufs=4, space="PSUM") as psA,
                ):
                    Wq1_s = pair_load(wA, "Wq1", Wq1_d, [128, DC, D], chunked)
                    Wk1_s = pair_load(wA, "Wk1", Wk1_d, [128, DC, D], chunked)
                    Wv1_s = pair_load(wA, "Wv1", Wv1_d, [128, DC, D], chunked)

                    for b in range(NG):
                        xth = xtp.tile([128, DC, 512], BF16, tag="xth")
                        xtl = xtp.tile([128, DC, 512], BF16, tag="xtl")
                        for t_, d_ in ((xth, XT_d[0]), (xtl, XT_d[1])):
                            nc.sync.dma_start(
                                t_[:],
                                d_[:, b * 512 : (b + 1) * 512].rearrange(
                                    "(c p) n -> p c n", p=128
                                ),
                            )
                        for W_s, (dsth, dstl) in (
                            (Wq1_s, (q1Th, q1Tl)),
                            (Wk1_s, (k1Th, k1Tl)),
                        ):
                            for c in range(DC):
                                ps = psA.tile([128, 512], F32, tag="psA")
                                for dc in range(DC):
                                    cs = slice(c * 128, (c + 1) * 128)
                                    mm3(
                                        ps[:],
                                        W_s[0][:, dc, cs],
                                        W_s[1][:, dc, cs],
                                        xth[:, dc, :],
                                        xtl[:, dc, :],
                                        start=(dc == 0),
                                        stop=(dc == DC - 1),
                                    )
                                bs = slice(b * 512, (b + 1) * 512)
                                split_evac(dsth[:, c, bs], dstl[:, c, bs], ps[:])
                        for lt in range(4):
                            t = b * 4 + lt
                            ps = psA.tile([128, 512], F32, tag="psA")
                            for dc in range(DC):
                                ls = slice(lt * 128, (lt + 1) * 128)
                                mm3(
                                    ps[:],
                                    xth[:, dc, ls],
                                    xtl[:, dc, ls],
                                    Wv1_s[0][:, dc, :],
                                    Wv1_s[1][:, dc, :],
                                    start=(dc == 0),
                                    stop=(dc == DC - 1),
                                )
                            split_evac(v1h[:, t, :], v1l[:, t, :], ps[:])

                # ---------------- phase B ----------------
                # Software-pipelined: per key-tile, the att0T accumulation for
                # tile t-1 is issued after the s1T matmuls for tile t, so the
                # PE never waits on the exp+split chain. Each group's attT/hT/
                # thresh matmuls (tail_pe) are deferred under the next group's
                # matmul stream; the normalization chain (tail_dve) runs
                # entirely on DVE+GPSIMD.
                with (
                    tc.tile_pool(name="phB", bufs=1) as phB,
                    tc.tile_pool(name="etp", bufs=3) as etp,
                    tc.tile_pool(name="psS", bufs=4, space="PSUM") as psS,
                    tc.tile_pool(name="psAcc", bufs=1, space="PSUM") as psAcc,
                ):
                    a0h = phB.tile([128, DC, 512], BF16, tag="a0h")
                    a0l = phB.tile([128, DC, 512], BF16, tag="a0l")

                    def tail_pe(g):
                        gs = slice(g * 512, (g + 1) * 512)
                        # attT[:, c, gs] = Wo1^T @ att0T_norm
                        for c in range(DC):
                            ps = psS.tile([128, 512], F32, tag="psS", name=f"attT{g}{c}")
                            for dc in range(DC):
                                cs = slice(c * 128, (c + 1) * 128)
                                mm3(
                                    ps[:],
                                    Wo1_s[0][:, dc, cs],
                                    Wo1_s[1][:, dc, cs],
                                    a0h[:, dc, :],
                                    a0l[:, dc, :],
                                    start=(dc == 0),
                                    stop=(dc == DC - 1),
                                )
                            split_evac(attTh[:, c, gs], attTl[:, c, gs], ps[:])
                        # hT = leaky(W1^T attT + b1)
                        ps_h = psS.tile([128, 512], F32, tag="psS", name=f"psh{g}")
                        for dc in range(DC):
                            mm3(
                                ps_h[:],
                                W1_s[0][:, dc, :],
                                W1_s[1][:, dc, :],
                                attTh[:, dc, gs],
                                attTl[:, dc, gs],
                                start=(dc == 0),
                                stop=False,
                            )
                        nc.tensor.matmul(
                            ps_h[:], b1r_s[0][:], ones[0:1, :], start=False, stop=False
                        )
                        nc.tensor.matmul(
                            ps_h[:], b1r_s[1][:], ones[0:1, :], start=False, stop=True
                        )
                        htmp = phB.tile([128, 512], F32, tag="htmp")
                        nc.vector.tensor_scalar_mul(htmp[:], ps_h[:], 0.1)
                        hf = phB.tile([128, 512], F32, tag="hf")
                        nc.vector.tensor_tensor(hf[:], ps_h[:], htmp[:], ALU.max)
                        hTh = phB.tile([128, 512], BF16, tag="hTh")
                        hTl = phB.tile([128, 512], BF16, tag="hTl")
                        split_evac(hTh[:], hTl[:], hf[:])
                        # threshneg row = W2n^T hT + b2n   [1, 512]
                        ps_t = psS.tile([128, 512], F32, tag="psS", name=f"pst{g}")
                        mm3(
                            ps_t[0:1, :], W2n_s[0][:], W2n_s[1][:], hTh[:], hTl[:],
                            start=True, stop=False,
                        )
                        nc.tensor.matmul(
                            ps_t[0:1, :], b2n_s[0][:], ones[0:1, :],
                            start=False, stop=False,
                        )
                        nc.tensor.matmul(
                            ps_t[0:1, :], b2n_s[1][:], ones[0:1, :],
                            start=False, stop=True,
                        )
                        trow = phB.tile([1, 512], F32, tag="trow")
                        nc.vector.tensor_copy(trow[:], ps_t[0:1, :])
                        for lt in range(4):
                            nc.sync.dma_start(
                                threshneg[:, g * 4 + lt : g * 4 + lt + 1],
                                trow[0:1, lt * 128 : (lt + 1) * 128],
                            )

                    for g in range(NG):
                        gs = slice(g * 512, (g + 1) * 512)
                        acc = psAcc.tile([128, DC, 512], F32, tag="acc")
                        rs = psS.tile([128, 512], F32, tag="psS", name=f"rs{g}")
                        pend = None  # (eh, el) awaiting att0T accumulation

                        def acc_tile(t, eh, el):
                            for c in range(DC):
                                cs = slice(c * 128, (c + 1) * 128)
                                mm3(
                                    acc[:, c, :],
                                    v1h[:, t, cs],
                                    v1l[:, t, cs],
                                    eh[:],
                                    el[:],
                                    start=(t == 0),
                                    stop=(t == NT - 1),
                                )
                            # row-sums: ones is exact in bf16, 2 passes
                            nc.tensor.matmul(
                                rs[0:1, :], ones[:, 0:1], eh[:],
                                start=(t == 0), stop=False,
                            )
                            nc.tensor.matmul(
                                rs[0:1, :], ones[:, 0:1], el[:],
                                start=False, stop=(t == NT - 1),
                            )

                        for t in range(NT):
                            ps = psS.tile([128, 512], F32, tag="psS", name=f"s1T{g}{t}")
                            for dc in range(DC):
                                ks = slice(t * 128, (t + 1) * 128)
                                mm3(
                                    ps[:],
                                    k1Th[:, dc, ks],
                                    k1Tl[:, dc, ks],
                                    q1Th[:, dc, gs],
                                    q1Tl[:, dc, gs],
                                    start=(dc == 0),
                                    stop=(dc == DC - 1),
                                )
                            ef = etp.tile([128, 512], F32, tag="ef")
                            nc.scalar.activation(ef[:], ps[:], AF.Exp)
                            eh = etp.tile([128, 512], BF16, tag="eh")
                            el = etp.tile([128, 512], BF16, tag="el")
                            split_evac(eh[:], el[:], ef[:])
                            if pend is not None:
                                acc_tile(t - 1, *pend)
                            pend = (eh, el)
                        acc_tile(NT - 1, *pend)

                        # previous group's deferred PE tail runs under this
                        # group's stream shadow
                        if g > 0:
                            tail_pe(g - 1)

                        # normalization chain: DVE + GPSIMD only (no PE)
                        recf = phB.tile([1, 512], F32, tag="recf")
                        nc.vector.reciprocal(recf[:], rs[0:1, :])
                        rbc = phB.tile([128, 512], F32, tag="rbc")
                        nc.gpsimd.partition_broadcast(rbc[:], recf[:])
                        a0f = phB.tile([128, DC, 512], F32, tag="a0f")
                        for c in range(DC):
                            nc.vector.tensor_mul(a0f[:, c, :], acc[:, c, :], rbc[:])
                            split_evac(a0h[:, c, :], a0l[:, c, :], a0f[:, c, :])
                    tail_pe(NG - 1)

            # ---------------- phase C ----------------
            with tc.tile_pool(name="qk2", bufs=1) as qk2:
                Wq2_s = pair_load(qk2, "Wq2", Wq2_d, [128, DC, D], chunked)
                Wk2_s = pair_load(qk2, "Wk2", Wk2_d, [128, DC, D], chunked)
                q2Th, q2Tl = pair_tile(qk2, "q2T", [128, DC, N])
                k2Th, k2Tl = pair_tile(qk2, "k2T", [128, DC, N])
                with tc.tile_pool(name="psC", bufs=4, space="PSUM") as psC:
                    for b in range(NG):
                        bs = slice(b * 512, (b + 1) * 512)
                        for W_s, (dsth, dstl) in (
                            (Wq2_s, (q2Th, q2Tl)),
                            (Wk2_s, (k2Th, k2Tl)),
                        ):
                            for c in range(DC):
                                ps = psC.tile([128, 512], F32, tag="psC")
                                for dc in range(DC):
                                    cs = slice(c * 128, (c + 1) * 128)
                                    mm3(
                                        ps[:],
                                        W_s[0][:, dc, cs],
                                        W_s[1][:, dc, cs],
                                        attTh[:, dc, bs],
                                        attTl[:, dc, bs],
                                        start=(dc == 0),
                                        stop=(dc == DC - 1),
                                    )
                                split_evac(dsth[:, c, bs], dstl[:, c, bs], ps[:])

                # ---------------- phase D ----------------
                with (
                    tc.tile_pool(name="phD", bufs=2) as phD,
                    tc.tile_pool(name="psD", bufs=2, space="PSUM") as psD,
                ):
                    for qt in range(NT):
                        qs = slice(qt * 128, (qt + 1) * 128)
                        s2 = psD.tile([128, NG, 512], F32, tag="s2")
                        for b in range(NG):
                            for dc in range(DC):
                                mm3(
                                    s2[:, b, :],
                                    q2Th[:, dc, qs],
                                    q2Tl[:, dc, qs],
                                    k2Th[:, dc, b * 512 : (b + 1) * 512],
                                    k2Tl[:, dc, b * 512 : (b + 1) * 512],
                                    start=(dc == 0),
                                    stop=(dc == DC - 1),
                                )
                        surv = phD.tile([128, NG, 512], F32, tag="surv")
                        denom = phD.tile([128, 1], F32, tag="denom")
                        nc.scalar.activation(
                            surv[:],
                            s2[:],
                            AF.Relu,
                            bias=threshneg[:, qt : qt + 1],
                            accum_out=denom[:],
                        )
                        recip = phD.tile([128, 1], F32, tag="recipD")
                        nc.vector.tensor_scalar_add(denom[:], denom[:], 1e-9)
                        nc.vector.reciprocal(recip[:], denom[:])
                        ot = phD.tile([128, NG, 512], F32, tag="ot")
                        nc.vector.tensor_scalar_mul(ot[:], surv[:], recip[:])
                        nc.sync.dma_start(
                            out_d[qs, :], ot.rearrange("p b n -> p (b n)")
                        )

    nc.finalize()
    return nc


_NC = None


def _get_nc():
    global _NC
    if _NC is None:
        _NC = build()
    return _NC


def make_in_maps(X, Wq1, Wk1, Wv1, Wo1, Wq2, Wk2, W1, b1, W2, b2):
    scale = np.float32(1.0 / np.sqrt(np.float32(D)))
    shared = {}

    def add_pair(name, x):
        h, l = split_pair(x)
        shared[name + "h"] = h
        shared[name + "l"] = l

    add_pair("Wq1", np.asarray(Wq1, np.float32) * scale)
    add_pair("Wk1", Wk1)
    add_pair("Wv1", Wv1)
    add_pair("Wo1", Wo1)
    add_pair("Wq2", np.asarray(Wq2, np.float32) * scale)
    add_pair("Wk2", Wk2)
    add_pair("W1", W1)
    add_pair("W2n", -np.asarray(W2, np.float32).reshape(CF, 1))
    add_pair("b1r", np.asarray(b1, np.float32).reshape(1, CF))
    add_pair("b2n", -np.asarray(b2, np.float32).reshape(1, 1))
    shared["ones"] = np.ones((128, 512), dtype=BFNP)

    X = np.asarray(X, dtype=np.float32)
    in_maps = []
    for i in range(B):
        xh, xl = split_pair(X[i].T)
        in_maps.append({"XTh": xh, "XTl": xl, **shared})
    return in_maps


def run(in_maps, trace=False, **kwargs):
    return run_bass_kernel_spmd(
        _get_nc(), in_maps, core_ids=list(range(B)), trace=trace, **kwargs
    )


def kernel(X, Wq1, Wk1, Wv1, Wo1, Wq2, Wk2, W1, b1, W2, b2):
    in_maps = make_in_maps(X, Wq1, Wk1, Wv1, Wo1, Wq2, Wk2, W1, b1, W2, b2)
    res = run(in_maps, trace=False)
    return np.stack([res.results[i]["out"] for i in range(B)], axis=0)
